# revision 29
# baseline (speedup 1.0000x reference)
"""Trainium2 Bass kernel for nn_ActorGraphPolicy (GNN message passing).

Data-parallel across 8 NeuronCores: each core handles 2048 of the 16384
batch rows. Feature-major on-chip layout (features on partitions, batch on
the free dim) so the tiny shared MLP weights are matmul-stationary and all
biases ride the ACT/DVE ops for free.

Per core:
  prep : cast state f32->bf16 (SWDGE cast DMA), X-bar DMA-transpose into
         feature-major xT pairs (two tree levels share 128 partitions).
  BU   : bottom-up scan, leaf (level 19) first. The scan-independent
         h = normalize(x @ uW1 + b1) path is emitted per level-pair with
         base-partition packed matmuls; the sequential message chain runs
         as two 1024-wide half-chains. tanh(m(l)) is spilled to DRAM for
         the top-down phase.
  TD   : top-down action MLPs, 4 x 512-wide chunk pipelines; action head
         rows packed into one PSUM tile per step; final tanh + 32x32
         block transpose + strided DMA to (batch, level) layout.
"""

import contextlib

import numpy as np

import concourse.bass as bass
import concourse.tile as tile
from concourse import bacc, mybir
from concourse.bass_utils import run_bass_kernel_spmd

F32 = mybir.dt.float32
BF16 = mybir.dt.bfloat16
AF = mybir.ActivationFunctionType
ALU = mybir.AluOpType
I32 = mybir.dt.int32

N_CORES = 8
B, L, S, MSG, HID = 16384, 20, 64, 64, 256
BC = B // N_CORES          # batch per core (2048)
NBLK = BC // 128           # 128-row batch blocks (16)
NPAIR = L // 2             # level pairs (10)
HALF = BC // 2             # BU half width (1024)
CH = 512                   # TD chunk width
NCH = BC // CH             # TD chunks (4)

WNAMES = [
    "uW1", "ub1", "uW2", "ub2", "uW3", "ub3",
    "aW1", "ab1", "aW2", "ab2", "aW3", "ab3",
    "mW1", "mb1", "mW2", "mb2", "mW3", "mb3",
]


def _build(nc: bass.Bass):
    state = nc.dram_tensor("state", [BC, L, S], F32, kind="ExternalInput")
    w = {n: nc.dram_tensor(n, shp, F32, kind="ExternalInput")
         for n, shp in [
             ("uW1", [S, 64]), ("ub1", [64]), ("uW2", [64 + MSG, 64]),
             ("ub2", [64]), ("uW3", [64, MSG]), ("ub3", [MSG]),
             ("aW1", [2 * MSG, HID]), ("ab1", [HID]), ("aW2", [HID, HID]),
             ("ab2", [HID]), ("aW3", [HID, 1]), ("ab3", [1]),
             ("mW1", [2 * MSG, HID]), ("mb1", [HID]), ("mW2", [HID, HID]),
             ("mb2", [HID]), ("mW3", [HID, MSG]), ("mb3", [MSG]),
         ]}
    out_ext = nc.dram_tensor("out", [BC, L], F32, kind="ExternalOutput")
    msdram = nc.dram_tensor("msdram", [L, MSG, BC], BF16)
    import os
    dbg = {}
    if os.environ.get("KDBG"):
        dbg["xt9"] = nc.dram_tensor("dbg_xt9", [128, BC], BF16,
                                    kind="ExternalOutput")
        dbg["hn"] = nc.dram_tensor("dbg_hn", [L, 64, BC], BF16,
                                   kind="ExternalOutput")
        dbg["ms"] = nc.dram_tensor("dbg_ms", [L, MSG, BC], BF16,
                                   kind="ExternalOutput")

    with tile.TileContext(nc) as tc:
        _emit(tc, nc, state, w, out_ext, msdram, dbg)
    return nc


def _emit(tc, nc, state, w, out_ext, msdram, dbg=None):
    ctx = contextlib.ExitStack()

    # ---------------- persistent SBUF pools ----------------
    pw = ctx.enter_context(tc.tile_pool(name="weights", bufs=1))
    pxbu = ctx.enter_context(tc.tile_pool(name="xbu", bufs=4))
    pms = ctx.enter_context(tc.tile_pool(name="ms", bufs=2))
    pact = ctx.enter_context(tc.tile_pool(name="act", bufs=1))
    pwork = ctx.enter_context(tc.tile_pool(name="work", bufs=2))
    pbig = ctx.enter_context(tc.tile_pool(name="bigwork", bufs=2))
    ptd = ctx.enter_context(tc.tile_pool(name="tdwork", bufs=2))
    pmdn = ctx.enter_context(tc.tile_pool(name="mdn", bufs=2))

    # ---------------- weights / constants ----------------
    def dup64(name):
        t = pw.tile([128, 64], BF16, tag=name, name=name)
        ap = w[name].ap()
        nc.gpsimd.dma_start(t[0:64, :], ap[:, :])
        nc.gpsimd.dma_start(t[64:128, :], ap[:, :])
        return t

    def dupbias(name):
        t = pw.tile([128, 1], F32, tag=name, name=name)
        ap = w[name].ap()[:, None]
        nc.gpsimd.dma_start(t[0:64, :], ap[:, :])
        nc.gpsimd.dma_start(t[64:128, :], ap[:, :])
        return t

    uW1d = dup64("uW1")
    uW3d = dup64("uW3")
    uW2t = pw.tile([128, 64], BF16, tag="uW2")
    nc.gpsimd.dma_start(uW2t[:, :], w["uW2"].ap()[:, :])
    ub1d = dupbias("ub1")
    ub2d = dupbias("ub2")
    ub3d = dupbias("ub3")

    # TD L1 weights with row halves swapped: TD xm tile is [md ; mu] while the
    # reference concat is [mu ; md].
    def w1perm(name):
        t = pw.tile([128, HID], BF16, tag=name + "p", name=name + "p")
        ap = w[name].ap()
        nc.gpsimd.dma_start(t[0:64, :], ap[64:128, :])
        nc.gpsimd.dma_start(t[64:128, :], ap[0:64, :])
        return t

    aW1p = w1perm("aW1")
    mW1p = w1perm("mW1")

    def ksplit(name, cols):
        ts = []
        for kh in range(2):
            t = pw.tile([128, cols], BF16, tag=f"{name}k{kh}", name=f"{name}k{kh}")
            nc.gpsimd.dma_start(t[:, :], w[name].ap()[kh * 128:(kh + 1) * 128, :])
            ts.append(t)
        return ts

    aW2k = ksplit("aW2", HID)
    mW2k = ksplit("mW2", HID)
    mW3k = ksplit("mW3", MSG)
    aW3k = ksplit("aW3", 1)

    def hbias(name):
        t0 = pw.tile([128, 1], F32, tag=name + "0", name=name + "0")
        t1 = pw.tile([128, 1], F32, tag=name + "1", name=name + "1")
        ap = w[name].ap()[:, None]
        nc.gpsimd.dma_start(t0[:, :], ap[0:128, :])
        nc.gpsimd.dma_start(t1[:, :], ap[128:256, :])
        return t0, t1

    ab1t = hbias("ab1")
    ab2t = hbias("ab2")
    mb1t = hbias("mb1")
    mb2t = hbias("mb2")
    mb3t = pw.tile([64, 1], F32, tag="mb3")
    nc.gpsimd.dma_start(mb3t[:, :], w["mb3"].ap()[:, None])
    ab3t = pw.tile([32, 1], F32, tag="ab3")
    nc.gpsimd.dma_start(ab3t[0:1, :], w["ab3"].ap()[:, None])
    nc.gpsimd.partition_broadcast(ab3t[:, :], ab3t[0:1, :], channels=32)

    onesm = pw.tile([128, 64], BF16, tag="onesm")
    nc.gpsimd.memset(onesm[:, :], 1.0)
    ident = pw.tile([128, 128], BF16, tag="ident")
    from concourse.masks import make_identity
    make_identity(nc, ident[:, :])

    a_store = pact.tile([32, BC], F32, tag="a_store")

    # xm tanh buffers, rotating: xbu[l][0:64] = tanh(h_n(l)),
    # xbu[l][64:128] = tanh(m(l+1)). Allocated lazily, bufs=4.
    xbu = {}

    def get_xbu(l):
        if l not in xbu:
            xbu[l] = pxbu.tile([128, BC], BF16, tag="xbu", name=f"xbu{l}")
        return xbu[l]

    # ---------------- BU phase ----------------
    bu_ctx = contextlib.ExitStack()
    ppA = bu_ctx.enter_context(tc.tile_pool(name="psA", bufs=1, space="PSUM"))
    ppN = bu_ctx.enter_context(tc.tile_pool(name="psN", bufs=1, space="PSUM"))
    ppB = bu_ctx.enter_context(tc.tile_pool(name="psB", bufs=2, space="PSUM"))
    ppTP = bu_ctx.enter_context(tc.tile_pool(name="psTP", bufs=1, space="PSUM"))

    st_blocks = state.ap().rearrange("(k p) l s -> k p (l s)", p=128)

    def emit_bu_a(p, xts):
        """h = normalize(x @ uW1 + b1) and its tanh, for levels 2p, 2p+1."""
        l0, l1 = 2 * p, 2 * p + 1
        hbw = pbig.tile([128, BC], BF16, tag="hbw", name="hbw")
        sqaw = pbig.tile([128, BC], BF16, tag="sqaw", name="sqaw", bufs=1)
        rvaw = pbig.tile([128, BC], F32, tag="rvaw", name="rvaw", bufs=1)
        for g in range(2):
            c0 = g * HALF
            gg = slice(c0, c0 + HALF)
            ha = ppA.tile([128, HALF], F32, tag="ha")
            for j in range(2):
                jj = slice(j * 512, (j + 1) * 512)
                cj = slice(c0 + j * 512, c0 + (j + 1) * 512)
                nc.tensor.matmul(ha[0:64, jj], uW1d[0:64, :], xts[p][0:64, cj])
                nc.tensor.matmul(ha[64:128, jj], uW1d[64:128, :],
                                 xts[p][64:128, cj])
            nc.vector.tensor_scalar_add(hbw[:, gg], ha[:, :], ub1d[:, 0:1])
            if g == 0:
                nc.vector.tensor_mul(sqaw[:, gg], hbw[:, gg], hbw[:, gg])
            else:
                nc.gpsimd.tensor_mul(sqaw[:, gg], hbw[:, gg], hbw[:, gg])
            for j in range(2):
                cj = slice(c0 + j * 512, c0 + (j + 1) * 512)
                nsq = ppN.tile([128, 512], F32, tag="nsq", bufs=1, name="nsq")
                nc.tensor.matmul(nsq[0:64, :], onesm[0:64, :], sqaw[0:64, cj])
                nc.tensor.matmul(nsq[64:128, :], onesm[64:128, :],
                                 sqaw[64:128, cj])
                nc.vector.reciprocal_approx_fast(rvaw[:, cj], nsq[:, :])
        bca = pbig.tile([128, BC], BF16, tag="bca", name="bca", bufs=1)
        nc.scalar.activation(bca[:, :], rvaw[:, :], AF.Sqrt)
        xaw = pbig.tile([128, BC], BF16, tag="xaw", name="xaw", bufs=1)
        nc.vector.tensor_mul(xaw[:, :], hbw[:, :], bca[:, :])
        nc.scalar.activation(get_xbu(l0)[0:64, :], xaw[0:64, :], AF.Tanh)
        nc.scalar.activation(get_xbu(l1)[0:64, :], xaw[64:128, :], AF.Tanh)

    def emit_bu_b(l):
        """One step of the sequential message chain (level l)."""
        X = get_xbu(l)
        ms_t = pms.tile([64, BC], BF16, tag="ms")
        mbw = pwork.tile([128, HALF], BF16, tag="mbw", name="mbw")
        for g in range(2):
            c0 = g * HALF
            gs = slice(g * 512, (g + 1) * 512)
            h2p = ppB.tile([128, 512], F32, tag="bps", bufs=2)
            nc.tensor.matmul(h2p[0:64, :], uW2t[:, :], X[:, c0:c0 + 512])
            nc.tensor.matmul(h2p[64:128, :], uW2t[:, :],
                             X[:, c0 + 512:c0 + 1024])
            h2s = pwork.tile([128, 512], BF16, tag="h2s")
            nc.scalar.activation(h2s[:, :], h2p[:, :], AF.Tanh,
                                 bias=ub2d[:, 0:1])
            msg = ppB.tile([128, 512], F32, tag="bps", bufs=2)
            nc.tensor.matmul(msg[0:64, :], uW3d[0:64, :], h2s[0:64, :])
            nc.tensor.matmul(msg[64:128, :], uW3d[64:128, :], h2s[64:128, :])
            nc.vector.tensor_scalar_add(mbw[:, gs], msg[:, :], ub3d[:, 0:1])
            sqm = pwork.tile([128, 512], BF16, tag="sqm")
            nc.scalar.activation(sqm[:, :], msg[:, :], AF.Square,
                                 bias=ub3d[:, 0:1])
            nsb = ppB.tile([128, 512], F32, tag="nsb")
            nc.tensor.matmul(nsb[0:64, :], onesm[0:64, :], sqm[0:64, :])
            nc.tensor.matmul(nsb[64:128, :], onesm[64:128, :], sqm[64:128, :])
            # rsqrt(nsb) on DVE: quake seed + 1 Newton step
            y0 = pwork.tile([128, 512], F32, tag="rsq_y0", name="y0")
            t1 = pwork.tile([128, 512], F32, tag="rsq_t1", name="t1")
            nc.vector.tensor_scalar(
                t1[:, :].bitcast(I32), nsb[:, :].bitcast(I32), 1, -1,
                op0=ALU.arith_shift_right, op1=ALU.bitwise_xor)
            nc.vector.tensor_scalar_add(y0[:, :].bitcast(I32),
                                        t1[:, :].bitcast(I32), 0x5f3759e0)
            w = pwork.tile([128, 512], F32, tag="rsq_w", name="w")
            nc.scalar.activation(w[:, :], y0[:, :], AF.Square)
            u = pwork.tile([128, 512], F32, tag="rsq_u", name="u")
            nc.vector.tensor_mul(u[:, :], w[:, :], nsb[:, :])
            v = pwork.tile([128, 512], F32, tag="rsq_v", name="v")
            nc.vector.tensor_scalar(v[:, :], u[:, :], -0.5, 1.5,
                                    op0=ALU.mult, op1=ALU.add)
            bcb = pwork.tile([128, 512], BF16, tag="rsq_o", name="bcb")
            nc.vector.tensor_mul(bcb[:, :], v[:, :], y0[:, :])
            nc.vector.tensor_mul(ms_t[:, c0:c0 + 512], mbw[0:64, gs],
                                 bcb[0:64, :])
            nc.vector.tensor_mul(ms_t[:, c0 + 512:c0 + 1024], mbw[64:128, gs],
                                 bcb[64:128, :])
        # tanh(m(l)) feeds step l-1 (same phase) and TD step l (via DRAM)
        Xn = get_xbu(l - 1)
        for q in range(4):
            qq = slice(q * 512, (q + 1) * 512)
            nc.scalar.activation(Xn[64:128, qq], ms_t[:, qq], AF.Tanh)
        nc.scalar.dma_start(msdram.ap()[l], Xn[64:128, :])

    # state view: [pair, partition(batch%128), block, 2*S contiguous values]
    st_pair = state.ap().rearrange("(k p) (lp w) v -> lp p k (w v)", p=128, w=2)

    with tc.tile_pool(name="xtpool", bufs=4) as pxt:

        def make_xt(p):
            xt = pxt.tile([128, BC], BF16, tag="xt", name=f"xt{p}")
            for kg in range(2):
                stg = pxt.tile([128, 8 * 2 * S], BF16, tag="stg", name="stg")
                nc.gpsimd.dma_start(
                    stg[:, :].rearrange("q (k u) -> q k u", k=8),
                    st_pair[p][:, 8 * kg:8 * (kg + 1)])
                tp = ppTP.tile([128, 1024], BF16, tag="tp", name="tp")
                for ki in range(8):
                    nc.tensor.transpose(tp[:, ki * 128:(ki + 1) * 128],
                                        stg[:, ki * 128:(ki + 1) * 128],
                                        ident[:, :])
                nc.vector.tensor_copy(xt[:, kg * 1024:(kg + 1) * 1024], tp[:, :])
            return xt

        xts = {NPAIR - 1: make_xt(NPAIR - 1), NPAIR - 2: make_xt(NPAIR - 2)}

        nc.gpsimd.memset(get_xbu(L - 1)[64:128, :], 0.0)  # tanh(m(20)) = 0
        for p in range(NPAIR - 1, -1, -1):
            emit_bu_a(p, xts)
            del xts[p]
            if p >= 2:
                xts[p - 2] = make_xt(p - 2)
            emit_bu_b(2 * p + 1)
            emit_bu_b(2 * p)
            del xbu[2 * p + 1]
        del xbu[0], xbu[-1]
        if dbg:
            nc.sync.dma_start(dbg["xt9"].ap()[:, :], xts[9][:, :])

    bu_ctx.close()
    if dbg:
        for l in range(L):
            nc.sync.dma_start(dbg["ms"].ap()[l], msdram.ap()[l])

    # ---------------- TD phase ----------------
    ppT = ctx.enter_context(tc.tile_pool(name="psT", bufs=1, space="PSUM"))
    ppL = {}
    for mlp in "am":
        for mh in range(2):
            ppL[(mlp, mh)] = ctx.enter_context(
                tc.tile_pool(name=f"psL{mlp}{mh}", bufs=1, space="PSUM"))
    ppMD = ctx.enter_context(tc.tile_pool(name="psMD", bufs=2, space="PSUM"))

    mdn_prev = None
    for l in range(L):
        X = pxbu.tile([128, BC], BF16, tag="xbu", name=f"xtd{l}")
        nc.sync.dma_start(X[64:128, :], msdram.ap()[l])
        if l == 0:
            nc.gpsimd.memset(X[0:64, :], 0.0)   # tanh(md(-1)) = 0
        aps = ppT.tile([97, 512], F32, tag="aps")
        mdn = pmdn.tile([64, BC], BF16, tag="mdn", name="mdn")
        mdbw = pmdn.tile([64, BC], BF16, tag="mdbw", name="mdbw")
        sqdw = pmdn.tile([64, BC], BF16, tag="sqdw", name="sqdw", bufs=1)
        rvtw = pmdn.tile([64, BC], F32, tag="rvtw", name="rvtw", bufs=1)
        for c in range(NCH):
            cc = slice(c * CH, (c + 1) * CH)
            if l > 0:
                nc.scalar.activation(X[0:64, cc], mdn_prev[:, cc], AF.Tanh)
            h1 = {}
            for mlp, W1p in (("a", aW1p), ("m", mW1p)):
                for mh in range(2):
                    ps = ppL[(mlp, mh)].tile([128, CH], F32, tag=f"L{mlp}{mh}",
                                             name=f"L{mlp}{mh}")
                    nc.tensor.matmul(ps[:, :], W1p[:, mh * 128:(mh + 1) * 128],
                                     X[:, cc])
                    hs = ptd.tile([128, CH], BF16, tag=f"h1{mlp}{mh}",
                                  name=f"h1{mlp}{mh}")
                    bias = (ab1t if mlp == "a" else mb1t)[mh]
                    if mlp == "a":
                        nc.scalar.activation(hs[:, :], ps[:, :], AF.Relu,
                                             bias=bias[:, 0:1])
                    else:
                        nc.vector.tensor_scalar(
                            hs[:, :], ps[:, :], bias[:, 0:1], 0.0,
                            op0=ALU.add, op1=ALU.max)
                    h1[(mlp, mh)] = hs
            h2 = {}
            for mlp, W2k in (("a", aW2k), ("m", mW2k)):
                for mh in range(2):
                    ps = ppL[(mlp, mh)].tile([128, CH], F32, tag=f"L{mlp}{mh}",
                                             name=f"L2{mlp}{mh}")
                    ms_ = slice(mh * 128, (mh + 1) * 128)
                    nc.tensor.matmul(ps[:, :], W2k[0][:, ms_],
                                     h1[(mlp, 0)][:, :], start=True, stop=False)
                    nc.tensor.matmul(ps[:, :], W2k[1][:, ms_],
                                     h1[(mlp, 1)][:, :], start=False, stop=True)
                    hs = ptd.tile([128, CH], BF16, tag=f"h2{mlp}{mh}",
                                  name=f"h2{mlp}{mh}")
                    bias = (ab2t if mlp == "a" else mb2t)[mh]
                    if mlp == "a":
                        nc.scalar.activation(hs[:, :], ps[:, :], AF.Relu,
                                             bias=bias[:, 0:1])
                    else:
                        nc.vector.tensor_scalar(
                            hs[:, :], ps[:, :], bias[:, 0:1], 0.0,
                            op0=ALU.add, op1=ALU.max)
                    h2[(mlp, mh)] = hs
            mdps = ppMD.tile([64, CH], F32, tag="mdps")
            nc.tensor.matmul(mdps[:, :], mW3k[0][:, :], h2[("m", 0)][:, :],
                             start=True, stop=False)
            nc.tensor.matmul(mdps[:, :], mW3k[1][:, :], h2[("m", 1)][:, :],
                             start=False, stop=True)
            nc.tensor.matmul(aps[32 * c:32 * c + 1, :], aW3k[0][:, :],
                             h2[("a", 0)][:, :], start=True, stop=False,
                             tile_position=(0, 32 * c))
            nc.tensor.matmul(aps[32 * c:32 * c + 1, :], aW3k[1][:, :],
                             h2[("a", 1)][:, :], start=False, stop=True,
                             tile_position=(0, 32 * c))
            nc.vector.tensor_scalar_add(mdbw[:, cc], mdps[:, :], mb3t[:, 0:1])
            nc.gpsimd.tensor_mul(sqdw[:, cc], mdbw[:, cc], mdbw[:, cc])
            nsqt = ppT.tile([64, CH], F32, tag="nsqt", bufs=1)
            nc.tensor.matmul(nsqt[:, :], onesm[0:64, :], sqdw[:, cc])
            nc.vector.reciprocal_approx_fast(rvtw[:, cc], nsqt[:, :])
            rst = ptd.tile([64, CH], BF16, tag="rst", name="rst", bufs=4)
            nc.scalar.activation(rst[:, :], rvtw[:, cc], AF.Sqrt)
            nc.vector.tensor_mul(mdn[:, cc], mdbw[:, cc], rst[:, :])
        mdn_prev = mdn
        asb = pwork.tile([97, 512], F32, tag="asb")
        nc.vector.tensor_copy(asb[:, :], aps[:, :])
        nc.scalar.dma_start(a_store[l:l + 1, :], asb[0:97:32, :])

    # ---------------- output: tanh, transpose, DMA ----------------
    att = pact.tile([32, BC], F32, tag="att")
    nc.gpsimd.memset(att[:, :], 0.0)
    nc.scalar.activation(att[0:20, :], a_store[0:20, :], AF.Tanh,
                         bias=ab3t[0:20, 0:1])
    otr = pact.tile([32, BC], F32, tag="otr")
    for k in range(NBLK):
        nc.vector.transpose(otr[:, k * 128:(k + 1) * 128],
                            att[:, k * 128:(k + 1) * 128])
    # otr[r, k*128 + 32*bj + c] = action(batch k*128 + 32*bj + r, level c)
    dst = out_ext.ap().rearrange("(k bj r) l -> r k bj l", r=32, bj=4)
    src = otr[:, :].rearrange("r (k bj c) -> r k bj c", bj=4, c=32)[:, :, :, 0:20]
    nc.sync.dma_start(dst, src)

    ctx.close()


_NC_CACHE = None


def _get_nc():
    global _NC_CACHE
    if _NC_CACHE is None:
        nc = bacc.Bacc("TRN2", target_bir_lowering=False, debug=False)
        _build(nc)
        nc.compile()
        _NC_CACHE = nc
    return _NC_CACHE


def kernel(**inputs) -> np.ndarray:
    nc = _get_nc()
    state = inputs["state"]
    in_maps = []
    for i in range(N_CORES):
        m = {"state": np.ascontiguousarray(state[i * BC:(i + 1) * BC])}
        for n in WNAMES:
            m[n] = np.ascontiguousarray(inputs[n])
        in_maps.append(m)
    res = run_bass_kernel_spmd(nc, in_maps, core_ids=list(range(N_CORES)))
    return np.concatenate([res.results[i]["out"] for i in range(N_CORES)], axis=0)
